# revision 41
# baseline (speedup 1.0000x reference)
"""Distributed Trainium2 kernel for nn_Attention_15710990369355.

Attention with QK-layernorm, sharded over 8 NeuronCores, collective-free:
  core c -> batch b = c // 4, head-group hg = c % 4 (4 of 16 heads).
  qkv weights column-sharded per head group; attention fully local;
  proj ROW-sharded (each core contracts only its own 256 head-rows) so
  each core emits a transposed partial out^T [1024, 2048]; the host sums
  the 4 partials per batch during unsharding (collectives on this stack
  cost ~60-100us each, far more than the host-side reduce).

Compute dtype: bf16 operands with fp32 PSUM accumulation; partial
outputs in fp16 (range is tiny, keeps output DMA at half cost).

Structure per core (b, hg):
  xt = x[b]^T (bf16, host-pretransposed)                  [1024, 2048]
  qkv: q|v and k natural via xt-stationary matmuls; LN means come free
       from host-appended per-head column-mean weight columns.
  LN:  var via square+reduce per tile, rsqrt batched per 4-tile group,
       fused (q-mu)*rstd evacuation.
  q̂,k̂ transposed on PE (gamma/beta fused into the PSUM evacuation).
  attention per 256-query block, all 4 heads at once:
       S^T row-packed matmul pairs (K=64 x2, tile_position packing),
       one exp per nk-tile (4 heads x 256 in one [128,1024] PSUM read),
       AV col-packed pairs into one shared PSUM bank,
       softmax sums via two M=1 ones-matmuls (two heads per N=512 row),
       1/s broadcast via a tiny K=33 select-matmul.
       Query-block 0 is interleaved into phase 1 to warm up ACT early.
  proj per 512-column group, pipelined behind attention; bias enters as
  pb/4 on every core (exact for the 4-way partial sum).
"""

import numpy as np
import ml_dtypes

import concourse.bass as bass
import concourse.mybir as mybir
import concourse.tile as tile
from concourse import bacc
from concourse.bass_utils import run_bass_kernel_spmd
from concourse.masks import make_identity

BF16 = mybir.dt.bfloat16
F16 = mybir.dt.float16
F32 = mybir.dt.float32
AF = mybir.ActivationFunctionType
ALU = mybir.AluOpType

B, N, C, H = 2, 2048, 1024, 16
Dh = C // H              # 64
HPC = 4                  # heads per core
NT = N // 128            # 16 row tiles
CK = C // 128            # 8 contraction chunks of the C dim
EPS = 1e-6
SCALE = Dh ** -0.5       # 0.125
NQB = 256                # query block size in attention
NQBS = N // NQB          # 8 query blocks
NCORES = 8
GROUPS = [[0, 1, 2, 3], [4, 5, 6, 7]]

nbf = ml_dtypes.bfloat16


def build(reps=1):
    nc = bacc.Bacc(
        "TRN2",
        target_bir_lowering=False,
        debug=False,
        enable_asserts=False,
        num_devices=NCORES,
    )

    # ---- dram parameters (per-core shards supplied by host) ----
    xt_d = nc.dram_tensor("xt", [C, N], BF16, kind="ExternalInput").ap()
    wqv_d = nc.dram_tensor("wqv", [C, 512], BF16, kind="ExternalInput").ap()
    wks_d = nc.dram_tensor("wks", [C, 264], BF16, kind="ExternalInput").ap()
    wp_d = nc.dram_tensor("wp", [256, C], BF16, kind="ExternalInput").ap()
    pb_d = nc.dram_tensor("pb", [128, 8], F32, kind="ExternalInput").ap()
    qgb_d = nc.dram_tensor("qgb", [128, 2], F32, kind="ExternalInput").ap()
    kgb_d = nc.dram_tensor("kgb", [128, 2], F32, kind="ExternalInput").ap()
    out_d = nc.dram_tensor("out", [C, N], F16, kind="ExternalOutput").ap()

    with tile.TileContext(nc) as tc:
        with (
            tc.tile_pool(name="singles", bufs=1) as singles,
            tc.tile_pool(name="psum_big", bufs=2, space="PSUM") as psum_big,
            tc.tile_pool(name="psum_one", bufs=4, space="PSUM") as psum_one,
            tc.tile_pool(name="work", bufs=3) as work,
            tc.tile_pool(name="pt_pool", bufs=4) as pt_pool,
            tc.tile_pool(name="small", bufs=4) as small,
            tc.tile_pool(name="dram", bufs=1, space="DRAM") as dram,
            tc.tile_pool(name="outp", bufs=3) as outp,
        ):
            for rep in range(reps):
                _emit(nc, tc, locals())

    nc.finalize()
    return nc


def _emit(nc, tc, env):
    singles = env["singles"]
    psum_big = env["psum_big"]
    psum_one = env["psum_one"]
    work = env["work"]
    pt_pool = env["pt_pool"]
    small = env["small"]
    dram = env["dram"]
    outp = env["outp"]
    xt_d, wqv_d, wks_d, wp_d, pb_d, qgb_d, kgb_d, out_d = (
        env["xt_d"], env["wqv_d"], env["wks_d"], env["wp_d"],
        env["pb_d"], env["qgb_d"], env["kgb_d"], env["out_d"],
    )

    # ---------------- load weights / constants ----------------
    wqv_s = singles.tile([128, CK, 512], BF16, name="wqv_s", tag="wqv_s")
    wqv_r = wqv_d.rearrange("(a p) n -> p a n", p=128)
    nc.sync.dma_start(out=wqv_s[:, 0:4, :], in_=wqv_r[:, 0:4, :])
    nc.sync.dma_start(out=wqv_s[:, 4:8, :], in_=wqv_r[:, 4:8, :])
    wks_s = singles.tile([128, CK, 264], BF16, name="wks_s", tag="wks_s")
    wks_r = wks_d.rearrange("(a p) n -> p a n", p=128)
    nc.sync.dma_start(out=wks_s[:, 0:4, :], in_=wks_r[:, 0:4, :])
    nc.sync.dma_start(out=wks_s[:, 4:8, :], in_=wks_r[:, 4:8, :])
    wp_s = singles.tile([128, 2, C], BF16, name="wp_s", tag="wp_s")
    nc.sync.dma_start(out=wp_s, in_=wp_d.rearrange("(a p) n -> p a n", p=128))
    pb4_s = singles.tile([128, 8], F32, name="pb4_s", tag="pb4_s")
    nc.sync.dma_start(out=pb4_s, in_=pb_d)
    qgb_s = singles.tile([128, 2], F32, name="qgb_s", tag="qgb_s")
    nc.sync.dma_start(out=qgb_s, in_=qgb_d)
    kgb_s = singles.tile([128, 2], F32, name="kgb_s", tag="kgb_s")
    nc.sync.dma_start(out=kgb_s, in_=kgb_d)

    ident = singles.tile([128, 128], BF16, name="ident", tag="ident")
    make_identity(nc, ident)
    ones128c = singles.tile([128, 1], BF16, name="ones128c", tag="ones128c")
    nc.vector.memset(ones128c, 1.0)
    eps_s = singles.tile([128, 1], F32, name="eps_s", tag="eps_s")
    nc.vector.memset(eps_s, EPS)
    # E: 1/s broadcast selector: row 0 -> out rows 0:64, row 32 -> 64:128
    e_sel = singles.tile([33, 128], BF16, name="e_sel", tag="e_sel")
    nc.vector.memset(e_sel, 0.0)
    nc.vector.memset(e_sel[0:1, 0:64], 1.0)
    nc.vector.memset(e_sel[32:33, 64:128], 1.0)

    # x^T, in 4 column-slices for pipelining
    xt_s = singles.tile([128, CK, N], BF16, name="xt_s", tag="xt_s")
    xt_r = xt_d.rearrange("(a p) n -> p a n", p=128)
    for i in range(4):
        sl = bass.ts(i, N // 4)
        nc.sync.dma_start(out=xt_s[:, :, sl], in_=xt_r[:, :, sl])

    # ---------------- persistent activations ----------------
    v_s = singles.tile([128, NT, HPC, Dh], BF16, name="v_s", tag="v_s")
    # q̂^T / k̂^T in head-pairs: [128=(2 heads x 64d), pair, n]
    # pair 0,1 = q head pairs (0,1),(2,3); pair 2,3 = k head pairs
    qkT = singles.tile([128, 4, N], BF16, name="qkT", tag="qkT")
    # attention output, transposed: chunk 0 = heads 0,1; chunk 1 = heads 2,3
    ot_s = singles.tile([128, 2, N], BF16, name="ot_s", tag="ot_s")
    qkraw_all = singles.tile([128, NT, 512], BF16, name="qkraw_all", tag="qkraw_all")
    mu_all = singles.tile([128, NT, 8], F32, name="mu_all", tag="mu_all")
    ssq_all = singles.tile([128, NT, 8], F32, name="ssq_all", tag="ssq_all")
    rstd_all = singles.tile([128, NT, 8], F32, name="rstd_all", tag="rstd_all")

    # ---------------- phase 1 + interleaved attention ----------------
    # PSUM layout of pq [128, 1024]: q [0:256], v [256:512] (bank0);
    # k [512:768], qmean [768:772], kmean [772:776] (bank1).
    PTSLOT = {0: 0, 1: 2, 2: 1, 3: 3}
    SROW = {0: 0, 1: 32, 2: 64, 3: 96}

    av_tiles = {}
    SCHRA_A = (2 ** 23) / np.log(2.0) * SCALE
    SCHRA_B = 127.0 * 2 ** 23 - 366393.0
    DVE_EXP_MS = set(range(NT - 3, NT))  # last 3 nk-tiles of each qb

    def attn_alloc(qb):
        av_ab = psum_one.tile([128, NQB], F32, name=f"av_ab_{qb}", tag="one")
        av_cd = psum_one.tile([128, NQB], F32, name=f"av_cd_{qb}", tag="one")
        s4 = psum_one.tile([33, 2 * NQB], F32, name=f"s4_{qb}", tag="one")
        av_tiles[qb] = (av_ab, av_cd, s4)

    def attn_chunk(qb, m):
        av_ab, av_cd, s4 = av_tiles[qb]
        qcols = bass.ts(qb, NQB)
        kcols = bass.ts(m, 128)
        st = psum_big.tile([128, 1024], F32, name=f"st_{qb}_{m}", tag="big")
        # row-packed pairs (A,B) then (C,D); column layout [A | C | B | D]
        nc.tensor.matmul(
            st[:, 0:NQB], qkT[0:64, 2, kcols], qkT[0:64, 0, qcols],
            start=True, stop=True,
        )
        nc.tensor.matmul(
            st[:, 512:512 + NQB], qkT[64:128, 2, kcols], qkT[64:128, 0, qcols],
            start=True, stop=True,
        )
        nc.tensor.matmul(
            st[:, NQB:512], qkT[0:64, 3, kcols], qkT[0:64, 1, qcols],
            start=True, stop=True,
        )
        nc.tensor.matmul(
            st[:, 512 + NQB:1024], qkT[64:128, 3, kcols], qkT[64:128, 1, qcols],
            start=True, stop=True,
        )
        pt = pt_pool.tile([128, 4, NQB], BF16, name=f"pt_{qb}_{m}", tag="pt")
        if m in DVE_EXP_MS:
            # exp on DVE (Schraudolph bit-trick): ACT is the bottleneck
            # engine in the attention phase; softmax ratios largely cancel
            # the ~2% approximation error.
            ib = work.tile([128, 4 * NQB], mybir.dt.int32, name=f"ib_{qb}_{m}", tag="ib")
            nc.vector.tensor_scalar(
                out=ib, in0=st, scalar1=SCHRA_A, scalar2=SCHRA_B,
                op0=ALU.mult, op1=ALU.add,
            )
            with nc.allow_low_precision(reason="approx exp, softmax cancels"):
                nc.vector.tensor_copy(
                    out=pt.rearrange("p a n -> p (a n)"),
                    in_=ib.bitcast(F32),
                )
        else:
            nc.scalar.activation(
                out=pt.rearrange("p a n -> p (a n)"), in_=st, func=AF.Exp,
                scale=SCALE,
            )
        # AV col-packed pairs
        nc.tensor.matmul(
            av_ab[0:64, :], v_s[:, m, 0, :], pt[:, 0, :],
            start=(m == 0), stop=(m == NT - 1), skip_group_check=True,
        )
        nc.tensor.matmul(
            av_ab[64:128, :], v_s[:, m, 1, :], pt[:, 2, :],
            start=(m == 0), stop=(m == NT - 1), skip_group_check=True,
        )
        nc.tensor.matmul(
            av_cd[0:64, :], v_s[:, m, 2, :], pt[:, 1, :],
            start=(m == 0), stop=(m == NT - 1), skip_group_check=True,
        )
        nc.tensor.matmul(
            av_cd[64:128, :], v_s[:, m, 3, :], pt[:, 3, :],
            start=(m == 0), stop=(m == NT - 1), skip_group_check=True,
        )
        # softmax sums: row 0 = [sA | sC], row 32 = [sB | sD]
        nc.tensor.matmul(
            s4[0:1, :], ones128c, pt[:, 0:2, :].rearrange("p a n -> p (a n)"),
            start=(m == 0), stop=(m == NT - 1),
            tile_position=(0, 0), skip_group_check=True,
        )
        nc.tensor.matmul(
            s4[32:33, :], ones128c, pt[:, 2:4, :].rearrange("p a n -> p (a n)"),
            start=(m == 0), stop=(m == NT - 1),
            tile_position=(0, 32), skip_group_check=True,
        )

    def attn_norm(qb):
        av_ab, av_cd, s4 = av_tiles.pop(qb)
        qcols = bass.ts(qb, NQB)
        rsc = small.tile([33, 2 * NQB], BF16, name=f"rsc_{qb}", tag="rsc")
        nc.vector.memset(rsc, 0.0)
        with nc.allow_low_precision(reason="1/s to bf16 for PE broadcast"):
            nc.vector.reciprocal(out=rsc[0:1, :], in_=s4[0:1, :])
            nc.vector.reciprocal(out=rsc[32:33, :], in_=s4[32:33, :])
        bc = psum_one.tile([128, 2 * NQB], F32, name=f"bc_{qb}", tag="one")
        nc.tensor.matmul(
            bc[:, 0:NQB], e_sel, rsc[0:33, 0:NQB],
            start=True, stop=True,
        )
        nc.tensor.matmul(
            bc[:, NQB:2 * NQB], e_sel, rsc[0:33, NQB:2 * NQB],
            start=True, stop=True,
        )
        rsb = small.tile([128, 2 * NQB], BF16, name=f"rsb_{qb}", tag="rsb")
        nc.vector.tensor_copy(out=rsb, in_=bc)
        nc.vector.tensor_mul(ot_s[:, 0, qcols], av_ab, rsb[:, 0:NQB])
        nc.vector.tensor_mul(ot_s[:, 1, qcols], av_cd, rsb[:, NQB:2 * NQB])

    def proj_group(g):
        # row-sharded proj partial (no collective): outT[oc, n] over this
        # core's 256 head-rows; host sums the 4 partials. Bias = pb/4 (exact).
        gcols = bass.ts(g, 512)
        for ot in range(8):
            occols = bass.ts(ot, 128)
            pp = psum_one.tile([128, 512], F32, name=f"pp_{g}_{ot}", tag="one")
            nc.tensor.matmul(
                pp, wp_s[:, 0, occols], ot_s[:, 0, gcols],
                start=True, stop=False,
            )
            nc.tensor.matmul(
                pp, wp_s[:, 1, occols], ot_s[:, 1, gcols],
                start=False, stop=True,
            )
            o_t = outp.tile([128, 512], F16, name=f"o_{g}_{ot}", tag="o")
            nc.vector.tensor_scalar(
                out=o_t, in0=pp, scalar1=pb4_s[:, ot:ot + 1], scalar2=None,
                op0=ALU.add,
            )
            nc.sync.dma_start(out=out_d[occols, gcols], in_=o_t)

    attn_alloc(0)
    for t in range(NT):
        ncols = bass.ts(t, 128)
        pq = psum_big.tile([128, 1024], F32, name=f"pq_{t}", tag="big")
        for c in range(CK):
            lhs = xt_s[:, c, ncols]
            nc.tensor.matmul(
                pq[:, 0:512], lhs, wqv_s[:, c, :],
                start=(c == 0), stop=(c == CK - 1),
            )
            nc.tensor.matmul(
                pq[:, 512:776], lhs, wks_s[:, c, :],
                start=(c == 0), stop=(c == CK - 1),
            )
        nc.scalar.copy(
            out=v_s[:, t, :, :],
            in_=pq[:, 256:512].rearrange("p (h d) -> p h d", h=HPC),
        )
        # raw q,k evac (bf16) on ACT (DVE is the phase-1 bottleneck)
        nc.scalar.copy(out=qkraw_all[:, t, 0:256], in_=pq[:, 0:256])
        nc.scalar.copy(out=qkraw_all[:, t, 256:512], in_=pq[:, 512:768])
        # per-tile LN stats: mean from matmul stat cols, var via square+reduce
        mu_t = mu_all[:, t, :]
        nc.vector.tensor_copy(out=mu_t, in_=pq[:, 768:776])
        sq = work.tile([128, 512], BF16, name=f"sq_{t}", tag="sq")
        nc.vector.tensor_mul(sq, qkraw_all[:, t, :], qkraw_all[:, t, :])
        nc.vector.reduce_sum(
            out=ssq_all[:, t, :],
            in_=sq.rearrange("p (g d) -> p g d", g=8),
            axis=mybir.AxisListType.X,
        )
        # rstd + normalize + transpose handled per 4-tile group below
        if t % 4 == 3:
            gi = t // 4
            gsl = slice(4 * gi, 4 * gi + 4)
            # var = ssq/64 + eps - mu^2  (batched over 4 tiles = [128, 32])
            msq = small.tile([128, 32], F32, name=f"msq_{gi}", tag="msq")
            nc.vector.tensor_mul(
                msq,
                mu_all[:, gsl, :].rearrange("p a b -> p (a b)"),
                mu_all[:, gsl, :].rearrange("p a b -> p (a b)"),
            )
            sc = small.tile([128, 32], F32, name=f"sc_{gi}", tag="sc")
            nc.vector.tensor_scalar(
                out=sc, in0=ssq_all[:, gsl, :].rearrange("p a b -> p (a b)"),
                scalar1=1.0 / Dh, scalar2=EPS, op0=ALU.mult, op1=ALU.add,
            )
            nc.vector.tensor_sub(sc, sc, msq)
            nc.scalar.activation(out=msq, in_=sc, func=AF.Sqrt)
            nc.vector.reciprocal(
                out=rstd_all[:, gsl, :].rearrange("p a b -> p (a b)"), in_=msq
            )
            for tt in range(4 * gi, 4 * gi + 4):
                tcols = bass.ts(tt, 128)
                qhat = work.tile([128, 512], BF16, name=f"qhat_{tt}", tag="qhat")
                for g in range(8):
                    nc.vector.tensor_scalar(
                        out=qhat[:, g * 64:(g + 1) * 64],
                        in0=qkraw_all[:, tt, g * 64:(g + 1) * 64],
                        scalar1=mu_all[:, tt, g:g + 1],
                        scalar2=rstd_all[:, tt, g:g + 1],
                        op0=ALU.subtract,
                        op1=ALU.mult,
                    )
                pt_ps = psum_one.tile([128, 512], BF16, name=f"tp_{tt}", tag="one")
                for p in range(4):
                    nc.tensor.transpose(
                        pt_ps[:, bass.ts(p, 128)], qhat[:, bass.ts(p, 128)], ident
                    )
                nc.vector.tensor_scalar(
                    out=qkT[:, 0:2, tcols],
                    in0=pt_ps[:, 0:256].rearrange("p (a n) -> p a n", a=2),
                    scalar1=qgb_s[:, 0:1], scalar2=qgb_s[:, 1:2],
                    op0=ALU.mult, op1=ALU.add,
                )
                nc.vector.tensor_scalar(
                    out=qkT[:, 2:4, tcols],
                    in0=pt_ps[:, 256:512].rearrange("p (a n) -> p a n", a=2),
                    scalar1=kgb_s[:, 0:1], scalar2=kgb_s[:, 1:2],
                    op0=ALU.mult, op1=ALU.add,
                )
            # interleave attention for qb 0 behind phase 1 (one-group lag)
            if gi >= 1:
                for m in range(4 * (gi - 1), 4 * gi):
                    attn_chunk(0, m)
    for m in range(12, NT):
        attn_chunk(0, m)
    attn_norm(0)

    # ---------------- remaining attention + pipelined proj ----------------
    for qb in range(1, NQBS):
        attn_alloc(qb)
        for m in range(NT):
            attn_chunk(qb, m)
        attn_norm(qb)
        if qb % 2 == 1:
            proj_group(qb // 2)

_CACHE = {}


def _shard_inputs(x, qkv_w, q_gamma, q_beta, k_gamma, k_beta, proj_w, proj_b):
    w = np.asarray(qkv_w, np.float32).reshape(C, 3, H, Dh)
    pw = np.asarray(proj_w, np.float32)
    pb = np.asarray(proj_b, np.float32)
    x = np.asarray(x, np.float32)

    def gb(gamma, beta):
        g2 = np.concatenate([np.asarray(gamma, np.float32)] * 2)
        b2 = np.concatenate([np.asarray(beta, np.float32)] * 2)
        return np.ascontiguousarray(np.stack([g2, b2], axis=1))

    qgb = gb(q_gamma, q_beta)
    kgb = gb(k_gamma, k_beta)

    in_maps = []
    for core in range(NCORES):
        b, hg = divmod(core, 4)
        hs = slice(4 * hg, 4 * hg + 4)
        wq = w[:, 0, hs, :].reshape(C, 256)
        wk = w[:, 1, hs, :].reshape(C, 256)
        wv = w[:, 2, hs, :].reshape(C, 256)
        wqv = np.concatenate([wq, wv], axis=1)
        wks = np.concatenate(
            [wk, wq.reshape(C, 4, 64).mean(-1), wk.reshape(C, 4, 64).mean(-1)],
            axis=1)
        in_maps.append({
            "xt": np.ascontiguousarray(x[b].T).astype(nbf),
            "wqv": np.ascontiguousarray(wqv).astype(nbf),
            "wks": np.ascontiguousarray(wks).astype(nbf),
            "wp": np.ascontiguousarray(pw[256 * hg:256 * (hg + 1), :]).astype(nbf),
            "pb": np.ascontiguousarray((pb / 4.0).reshape(8, 128).T).astype(np.float32),
            "qgb": qgb,
            "kgb": kgb,
        })
    return in_maps


def run(inputs, trace=False, **kw):
    if "nc" not in _CACHE:
        _CACHE["nc"] = build()
    nc = _CACHE["nc"]
    in_maps = _shard_inputs(**inputs)
    try:
        res = run_bass_kernel_spmd(
            nc, in_maps, core_ids=list(range(NCORES)), trace=trace, **kw
        )
    except ModuleNotFoundError:
        # axon NTFF profile hook not shipped in this container; fall back to
        # an untraced run rather than crashing when BASS_TRACE is set.
        import os
        os.environ["BASS_NEVER_TRACE"] = "1"
        res = run_bass_kernel_spmd(
            nc, in_maps, core_ids=list(range(NCORES)), trace=False, **kw
        )
    out = np.empty((B, N, C), np.float32)
    for b in range(B):
        acc = np.zeros((C, N), np.float32)
        for hg in range(4):
            acc += res.results[4 * b + hg]["out"].astype(np.float32)
        out[b] = acc.T
    return out, res


def kernel(**inputs) -> np.ndarray:
    out, _ = run(inputs)
    return out


# ---------------------------------------------------------------------------
# timing apparatus (dev only): the container has no NTFF profiling, so device
# time is estimated from wall-clock slopes of async-pipelined executions,
# differencing reps=1 vs reps=K NEFFs (per-call overhead cancels).
# ---------------------------------------------------------------------------

def _make_runner(nc, in_maps):
    import jax
    import jax.numpy as jnp
    from jax.experimental.shard_map import shard_map
    from jax.sharding import Mesh, NamedSharding, PartitionSpec
    import concourse.mybir as mybir_
    from concourse import bass2jax

    bass2jax.install_neuronx_cc_hook()

    in_names, out_names, out_avals = [], [], []
    partition_name = (
        nc.partition_id_tensor.name if nc.partition_id_tensor else None
    )
    for alloc in nc.m.functions[0].allocations:
        if not isinstance(alloc, mybir_.MemoryLocationSet):
            continue
        name = alloc.memorylocations[0].name
        if alloc.kind == "ExternalInput":
            if name != partition_name:
                in_names.append(name)
        elif alloc.kind == "ExternalOutput":
            out_names.append(name)
            out_avals.append(
                jax.core.ShapedArray(
                    tuple(alloc.tensor_shape), mybir_.dt.np(alloc.dtype)
                )
            )
    n_params = len(in_names)
    all_in_names = in_names + out_names
    if partition_name is not None:
        all_in_names.append(partition_name)

    def _body(*args):
        operands = list(args)
        if partition_name is not None:
            operands.append(bass2jax.partition_id_tensor())
        outs = bass2jax._bass_exec_p.bind(
            *operands,
            out_avals=tuple(out_avals),
            in_names=tuple(all_in_names),
            out_names=tuple(out_names),
            lowering_input_output_aliases=(),
            sim_require_finite=True,
            sim_require_nnan=True,
            nc=nc,
        )
        return tuple(outs)

    devices = jax.devices()[:NCORES]
    mesh = Mesh(np.asarray(devices), ("core",))
    sharded = jax.jit(
        shard_map(
            _body, mesh=mesh,
            in_specs=(PartitionSpec("core"),) * (n_params + len(out_names)),
            out_specs=(PartitionSpec("core"),) * len(out_names),
            check_rep=False,
        ),
        keep_unused=True,
    )
    sh = NamedSharding(mesh, PartitionSpec("core"))
    concat_in = [
        jax.device_put(
            np.concatenate([np.asarray(in_maps[c][nm]) for c in range(NCORES)], 0),
            sh,
        )
        for nm in in_names
    ]
    # zero "output seed" params: not donated (kernel writes every output
    # element), so the same device buffers are reused every call.
    concat_in += [
        jax.device_put(
            np.zeros((NCORES * a.shape[0],) + tuple(a.shape[1:]), a.dtype), sh
        )
        for a in out_avals
    ]

    def call_async():
        return sharded(*concat_in)

    def call():
        out = call_async()
        jax.block_until_ready(out)
        return out

    call.call_async = call_async
    return call


def measure_slope(reps=1, iters=10, lo_m=4, hi_m=24):
    """Median wall-clock slope (seconds per async-dispatched execution) of a
    build(reps=reps) executable. Run in its OWN process: two bass
    executables in one process desync the axon terminal."""
    import time
    import jax
    import sys as _sys
    if "/root/problem" not in _sys.path:
        _sys.path.insert(0, "/root/problem")
    import reference
    cpu = jax.devices("cpu")[0]
    with jax.default_device(cpu):
        inputs = {k: np.asarray(v) for k, v in reference.setup_inputs().items()}
    in_maps = _shard_inputs(**inputs)

    call = _make_runner(build(reps=reps), in_maps)
    call()  # warm up (compile + first exec)
    call()

    def pipeline(m):
        t0 = time.perf_counter()
        outs = [call.call_async() for _ in range(m)]
        jax.block_until_ready(outs)
        return time.perf_counter() - t0

    slopes = []
    for _ in range(iters):
        tl = pipeline(lo_m)
        th = pipeline(hi_m)
        slopes.append((th - tl) / (hi_m - lo_m))
    slopes.sort()
    print(f"reps={reps} slopes us: {[f'{s * 1e6:.1f}' for s in slopes]}")
    return slopes[len(slopes) // 2]
